# revision 1
# baseline (speedup 1.0000x reference)
"""Trainium2 Bass kernel for GAT-style attention softmax (CochainMessagePassing).

Computes, for inputs
    x       [4, 4, 1024, 512]  f32
    attn_w  [4, 4, 8, 1024, 128] f32
the output
    out     [4, 4, 1024, 8, 1024] f32
where per (b, n, head h):
    xh   = x[b, n, :, h*64:(h+1)*64]            # [1024, 64]
    a2   = attn_w[b, n, h, :, 64:128]           # [1024, 64]
    e    = a2 @ xh.T                            # [1024, 1024]
    out[b, n, i, h, j] = softmax_j(e_self[i] + e[i, j]) = softmax_j(e[i, j])
(e_self is constant along the softmax axis so it cancels; a1 is never needed).

Sharding: the 16 (b, n) slabs are split 2-per-core across 8 NeuronCores
(pure data parallel, no collectives).
"""

import sys

sys.path.insert(0, "/opt/trn_rl_repo")

from contextlib import ExitStack

import numpy as np

import concourse.bass as bass
import concourse.tile as tile
from concourse import mybir
from concourse.bass_utils import run_bass_kernel_spmd
from concourse.masks import make_identity

NUM_CORES = 8
SLABS_PER_CORE = 2  # (b, n) pairs per core
N_C = 1024  # complexes
D = 512
H = 8  # heads
DH = 64  # head dim
NIB = N_C // 128  # i-blocks per slab

F32 = mybir.dt.float32
F32R = mybir.dt.float32r

# score matmuls in float32r (full 4-byte operands, 1 cycle/row for N>=256)
USE_F32R = True
BF16 = mybir.dt.bfloat16

# Engine-isolation probe for bench builds (no NTFF traces under this axon
# client). One of: "" (full), "noact" (skip exp), "noout" (skip output DMA),
# "nodve" (skip accum/recip/mul, DMA the exp tile), "nomm" (skip matmuls).
PROBE = ""

# A/B knob (cost-model makespan preferred per-head output DMAs over
# [128, 2048] head-pair batched ones; kept for future experiments)
OUT_BATCH = False

# Where the softmax row-sum is accumulated:
#   "dve"   - DVE tensor_scalar copy-pass with accum_out (4x mode per model)
#   "act"   - ACT activation accum_out (costs an accumulator-read per exp)
#   "mixed" - alternate per i-block to balance both engines
ACCUM_MODE = "dve"

# Matmul operand dtype: "f32r" (exact, 1 cyc/row per cost model) or "fp16"
# (hedge in case f32r's fast path doesn't hold on real HW; ~0.8% out err)
MM_DTYPE = "f32r"


def make_pools(ctx: ExitStack, tc: tile.TileContext):
    nc = tc.nc
    pools = {}
    pools["const"] = ctx.enter_context(tc.tile_pool(name="const", bufs=1))
    pools["xstage"] = ctx.enter_context(tc.tile_pool(name="xstage", bufs=2))
    pools["xT"] = ctx.enter_context(tc.tile_pool(name="xT", bufs=2))
    pools["a2stage"] = ctx.enter_context(tc.tile_pool(name="a2stage", bufs=2))
    pools["a2T"] = ctx.enter_context(tc.tile_pool(name="a2T", bufs=2))
    pools["exp"] = ctx.enter_context(tc.tile_pool(name="exp", bufs=16))
    pools["outp"] = ctx.enter_context(tc.tile_pool(name="outp", bufs=10))
    pools["stat"] = ctx.enter_context(tc.tile_pool(name="stat", bufs=4))
    pools["scratchp"] = ctx.enter_context(tc.tile_pool(name="scratchp", bufs=1))
    pools["tpsum"] = ctx.enter_context(tc.tile_pool(name="tpsum", bufs=2, space="PSUM"))
    pools["spsum"] = ctx.enter_context(tc.tile_pool(name="spsum", bufs=3, space="PSUM"))
    identity = pools["const"].tile([128, 128], F32)
    make_identity(nc, identity[:])
    pools["identity"] = identity
    # dead sink for the DVE accumulate pass (only accum_out matters)
    pools["scratch"] = pools["scratchp"].tile([128, N_C], BF16, name="scratch")
    if PROBE:
        # probe builds read tiles that are never produced; give them a
        # defined source so Tile dependency tracking stays happy
        nc.vector.memset(pools["scratch"][:], 1.0)
        pools["psrc"] = pools["scratchp"].tile(
            [128, N_C], BF16, name="psrc", tag="psrc"
        )
        nc.vector.memset(pools["psrc"][:], 1.0)
    return pools


def build_kernel_body(pools, tc: tile.TileContext, out_ap, x_ap, w_ap):
    nc = tc.nc
    if PROBE == "empty":
        return
    xstage = pools["xstage"]
    xT_pool = pools["xT"]
    a2stage = pools["a2stage"]
    a2T_pool = pools["a2T"]
    exp_pool = pools["exp"]
    outp = pools["outp"]
    stat_pool = pools["stat"]
    tpsum = pools["tpsum"]
    spsum = pools["spsum"]
    identity = pools["identity"]

    if MM_DTYPE == "fp16":
        mm_dt = mybir.dt.float16
    else:
        mm_dt = F32R if USE_F32R else F32

    def load_a2(s, q, split=False):
        # a2s[:, ib*128 + hh*64 + k] = attn_w[s, 2q+hh, ib*128 + i, 64 + k]
        # split=True loads i-halves as separate DMAs so the first i-block
        # transposes can start after a quarter of the bytes land
        a2s = a2stage.tile([128, N_C], F32, name="a2s", tag="a2s")
        a2s_r = a2s[:].rearrange("p (a c) -> p a c", c=128)
        halves = ((0, N_C // 2), (N_C // 2, N_C)) if split else ((0, N_C),)
        for i0, i1 in halves:
            a0, a1 = i0 // 128, i1 // 128
            for hh in range(2):
                h = 2 * q + hh
                src = w_ap[s, h, i0:i1, DH : 2 * DH].rearrange(
                    "(a p) k -> p a k", p=128
                )
                nc.gpsimd.dma_start(a2s_r[:, a0:a1, hh * DH : (hh + 1) * DH], src)
        return a2s

    a2s_next = None
    for s in range(SLABS_PER_CORE):
        # first head-pair's a2 load goes ahead of the bulk x load so the
        # a2 transposes (which gate the first scores) aren't queued behind
        # 2 MB of x on the DMA engines
        if a2s_next is None:
            a2s_next = load_a2(s, 0, split=True)
        # ---- x[s]: [1024 (j), 512 (d)] -> xT [512 (d), 1024 (j)] ----
        # xT packed as one [128, 4096] tile: xT[dd, p*1024 + j] = x[s, j, p*128+dd]
        # Loaded per 128-wide d-block (p): one DMA brings every j row's
        # p-th 512 B chunk.  The p-block's transposes are emitted inside
        # the q loop (p == q) so PE's in-order stream doesn't queue all 32
        # transposes ahead of the first score matmuls.
        xT = xT_pool.tile([128, 4 * N_C], mm_dt)
        xps = []
        for p in range(4):
            xp = xstage.tile([128, NIB * 128], F32, name="xp", tag="xp", bufs=6)
            # first-ever p-block in j-halves (startup fine-graining)
            halves = ((0, N_C // 2), (N_C // 2, N_C)) if (s == 0 and p == 0) else (
                (0, N_C),
            )
            for j0, j1 in halves:
                src = x_ap[s][j0:j1, p * 128 : (p + 1) * 128].rearrange(
                    "(a pp) d -> pp a d", pp=128
                )
                nc.gpsimd.dma_start(
                    xp[:, j0:j1].rearrange("pp (a d) -> pp a d", d=128), src
                )
            xps.append(xp)

        # ---- per head-pair q: heads (2q, 2q+1) ----
        for q in range(4):
            a2s = a2s_next
            # prefetch the next pair's a2 (next q, or q=0 of the next slab)
            if q < 3:
                a2s_next = load_a2(s, q + 1)
            elif s + 1 < SLABS_PER_CORE:
                a2s_next = load_a2(s + 1, 0)
            else:
                a2s_next = None

            # transpose to a2T [128 (k of pair), 1024 (i)]:
            # a2T[hh*64 + k, i] = a2 of head (2q+hh) at [i, k]
            # 4 transposes packed into one PSUM bank (only the first clears
            # the bank's has_written bits), drained by a single DVE copy.
            a2T = a2T_pool.tile([128, N_C], mm_dt)
            for ibq in range(NIB // 4):
                ps = tpsum.tile([128, 512], F32)
                for k in range(4):
                    ib = 4 * ibq + k
                    nc.tensor.matmul(
                        ps[:, k * 128 : (k + 1) * 128],
                        a2s[:, ib * 128 : (ib + 1) * 128],
                        identity[:],
                        is_transpose=True,
                        start=(k == 0),
                        stop=(k == 3),
                    )
                nc.vector.tensor_copy(
                    out=a2T[:, ibq * 512 : (ibq + 1) * 512], in_=ps[:]
                )

            # x transposes for this pair's d-block (p == q), same packing
            for jbq in range(NIB // 4):
                ps = tpsum.tile([128, 512], F32)
                for k in range(4):
                    jb = 4 * jbq + k
                    nc.tensor.matmul(
                        ps[:, k * 128 : (k + 1) * 128],
                        xps[q][:, jb * 128 : (jb + 1) * 128],
                        identity[:],
                        is_transpose=True,
                        start=(k == 0),
                        stop=(k == 3),
                    )
                nc.vector.tensor_copy(
                    out=xT[:, q * N_C + jbq * 512 : q * N_C + (jbq + 1) * 512],
                    in_=ps[:],
                )

            # ---- scores + softmax per head (two-phase over i-blocks) ----
            # Phase 1: scores -> exp (ACT, no accum read) + DVE accumulate
            # pass at 4x (bf16) collecting per-i row sums.  Phase 2: one
            # batched reciprocal, then DVE 4x normalize muls.  Output tiles
            # span the head pair ([128, 2048]); heads are adjacent in DRAM,
            # so one DMA per (q, ib) writes 4 KiB contiguous per partition
            # and halves the SP DGE-config cost per byte.
            outts = [
                outp.tile([128, 2 * N_C], BF16, name="outt", tag="outt")
                for _ in range(NIB)
            ] if OUT_BATCH else None
            for hh in range(2):
                h = 2 * q + hh
                # rhs: xT rows h*64..h*64+64 = partition offset hh*64 of block p=q
                rhs_all = xT[hh * DH : (hh + 1) * DH, q * N_C : (q + 1) * N_C]
                sums = stat_pool.tile([128, NIB], F32, tag="sums")
                rec = stat_pool.tile([128, NIB], F32, tag="rec")
                expts = []
                for ib in range(NIB):
                    lhsT = a2T[hh * DH : (hh + 1) * DH, ib * 128 : (ib + 1) * 128]
                    psc = spsum.tile([128, N_C], F32)
                    if PROBE != "nomm":
                        for jc in range(2):
                            nc.tensor.matmul(
                                psc[:, jc * 512 : (jc + 1) * 512],
                                lhsT,
                                rhs_all[:, jc * 512 : (jc + 1) * 512],
                                start=True,
                                stop=True,
                            )
                    on_act = ACCUM_MODE == "act" or (
                        ACCUM_MODE == "mixed" and ib % 2 == 0
                    )
                    if PROBE in ("noact", "nomm"):
                        expt = pools["psrc"]
                        on_act = False
                    else:
                        expt = exp_pool.tile([128, N_C], BF16)
                        if on_act and PROBE != "nodve":
                            nc.scalar.activation(
                                expt[:],
                                psc[:],
                                mybir.ActivationFunctionType.Exp,
                                accum_out=sums[:, ib : ib + 1],
                            )
                        else:
                            nc.scalar.activation(
                                expt[:], psc[:], mybir.ActivationFunctionType.Exp
                            )
                    if PROBE != "nodve" and not on_act:
                        nc.vector.tensor_scalar(
                            out=pools["scratch"][:],
                            in0=expt[:],
                            scalar1=1.0,
                            scalar2=None,
                            op0=mybir.AluOpType.mult,
                            op1=mybir.AluOpType.add,
                            accum_out=sums[:, ib : ib + 1],
                        )
                    expts.append(expt)
                if PROBE != "nodve":
                    # split so the first chunks' normalize muls can overlap
                    # the later accumulate passes; the final head uses
                    # quarters to shorten the end-of-kernel drain
                    last = s == SLABS_PER_CORE - 1 and q == 3 and hh == 1
                    step = NIB // 4 if last else NIB // 2
                    for c0 in range(0, NIB, step):
                        nc.vector.reciprocal(
                            rec[:, c0 : c0 + step], sums[:, c0 : c0 + step]
                        )
                for ib in range(NIB):
                    if PROBE == "nodve":
                        if PROBE != "noout":
                            # outts never written in this probe; ship exp tiles
                            nc.sync.dma_start(
                                out_ap[s, ib * 128 : (ib + 1) * 128, h, :],
                                expts[ib][:],
                            )
                        continue
                    if OUT_BATCH:
                        nc.vector.tensor_scalar_mul(
                            outts[ib][:, hh * N_C : (hh + 1) * N_C],
                            expts[ib][:],
                            rec[:, ib : ib + 1],
                        )
                        if hh == 1 and PROBE != "noout":
                            dst = out_ap[
                                s, ib * 128 : (ib + 1) * 128, 2 * q : 2 * q + 2, :
                            ].rearrange("p h j -> p (h j)")
                            nc.sync.dma_start(dst, outts[ib][:])
                    else:
                        outt = outp.tile(
                            [128, N_C], BF16, name="outs", tag="outs", bufs=8
                        )
                        nc.vector.tensor_scalar_mul(
                            outt[:], expts[ib][:], rec[:, ib : ib + 1]
                        )
                        if PROBE != "noout":
                            nc.sync.dma_start(
                                out_ap[s, ib * 128 : (ib + 1) * 128, h, :], outt[:]
                            )


def _split_multi_waits(nc):
    """walrus's per-instruction codegen structs hold only one embedded sync
    wait; hoist multi-wait instructions' waits onto standalone same-engine
    wait instructions placed immediately before them (program order on the
    sequencer preserves semantics)."""
    ctr = 0
    for f in nc.m.functions:
        for blk in f.blocks:
            out = []
            changed = False
            for inst in blk.instructions:
                tname = type(inst).__name__
                si = inst.sync_info
                if (
                    tname != "InstEventSemaphore"
                    and si is not None
                    and si.on_wait
                    and len(si.on_wait) > 1
                ):
                    for w in si.on_wait:
                        wi = mybir.InstEventSemaphore(name=f"WSPLIT-{ctr}")
                        ctr += 1
                        wi.engine = inst.engine
                        wi.sync_info = mybir.SyncInfo(on_wait=[w], on_update=[])
                        out.append(wi)
                    inst.sync_info = mybir.SyncInfo(
                        on_wait=[], on_update=list(si.on_update)
                    )
                    changed = True
                out.append(inst)
            if changed:
                blk.instructions = out
    return ctr


def build_bass(bench_repeats=None, split_waits=True):
    nc = bass.Bass("TRN2", target_bir_lowering=False, debug=False)
    if bench_repeats is None:
        x_ap = nc.dram_tensor(
            "x", [SLABS_PER_CORE, N_C, D], F32, kind="ExternalInput"
        ).ap()
        w_ap = nc.dram_tensor(
            "attn_w", [SLABS_PER_CORE, H, N_C, 2 * DH], F32, kind="ExternalInput"
        ).ap()
        out_ap = nc.dram_tensor(
            "out", [SLABS_PER_CORE, N_C, H, N_C], BF16, kind="ExternalOutput"
        ).ap()
        with tile.TileContext(nc) as tc:
            with ExitStack() as ctx:
                pools = make_pools(ctx, tc)
                build_kernel_body(pools, tc, out_ap, x_ap, w_ap)
    else:
        # bench variant: all big tensors are device-internal (no host I/O);
        # tiny external in/out keep the custom-call ABI happy. Internal
        # inputs are zeroed once, then the body runs `bench_repeats` times
        # (unrolled; For_i trips a walrus InstISA codegen bug).
        x_ap = nc.dram_tensor("xi", [SLABS_PER_CORE, N_C, D], F32).ap()
        w_ap = nc.dram_tensor("wi", [SLABS_PER_CORE, H, N_C, 2 * DH], F32).ap()
        out_ap = nc.dram_tensor("oi", [SLABS_PER_CORE, N_C, H, N_C], BF16).ap()
        tin = nc.dram_tensor("tin", [1, 4], F32, kind="ExternalInput").ap()
        tout = nc.dram_tensor("tout", [1, 4], F32, kind="ExternalOutput").ap()
        with tile.TileContext(nc) as tc:
            with ExitStack() as ctx:
                pools = make_pools(ctx, tc)
                tiny = pools["const"].tile([1, 4], F32)
                nc.gpsimd.dma_start(tiny[:], tin[:, :])
                nc.gpsimd.dma_start(tout[:, :], tiny[:])
                zt = pools["const"].tile([128, 4 * N_C], F32)
                nc.vector.memset(zt[:], 0.0)
                x_flat = x_ap.rearrange("s (a p) d -> (s a) p d", p=128)
                for t in range(x_flat.shape[0]):
                    nc.gpsimd.dma_start(x_flat[t], zt[:, :D])
                w_flat = w_ap.rearrange("s h (a p) k -> (s h a) p k", p=128)
                for t in range(w_flat.shape[0]):
                    nc.gpsimd.dma_start(w_flat[t], zt[:, : 2 * DH])
                for _ in range(bench_repeats):
                    build_kernel_body(pools, tc, out_ap, x_ap, w_ap)
    if split_waits:
        _split_multi_waits(nc)
    return nc


_NC_CACHE = None


def _get_nc():
    global _NC_CACHE
    if _NC_CACHE is None:
        _NC_CACHE = build_bass()
    return _NC_CACHE


def kernel(x: np.ndarray, attn_w: np.ndarray, _trace: bool = False):
    assert x.shape == (4, 4, N_C, D), x.shape
    assert attn_w.shape == (4, 4, H, N_C, 2 * DH), attn_w.shape
    xs = np.ascontiguousarray(x, dtype=np.float32).reshape(16, N_C, D)
    ws = np.ascontiguousarray(attn_w, dtype=np.float32).reshape(16, H, N_C, 2 * DH)
    in_maps = [
        {
            "x": np.ascontiguousarray(xs[2 * c : 2 * c + 2]),
            "attn_w": np.ascontiguousarray(ws[2 * c : 2 * c + 2]),
        }
        for c in range(NUM_CORES)
    ]
    nc = _get_nc()
    res = run_bass_kernel_spmd(
        nc, in_maps, core_ids=list(range(NUM_CORES)), trace=_trace
    )
    out = np.concatenate(
        [res.results[c]["out"].astype(np.float32) for c in range(NUM_CORES)], axis=0
    )
    if _trace:
        kernel.last_exec_time_ns = res.exec_time_ns
    return out.reshape(4, 4, N_C, H, N_C)


kernel.last_exec_time_ns = None



# revision 10
# speedup vs baseline: 350.8926x; 350.8926x over previous
"""Trainium2 Bass kernel for GAT-style attention softmax (CochainMessagePassing).

Computes, for inputs
    x       [4, 4, 1024, 512]  f32
    attn_w  [4, 4, 8, 1024, 128] f32
the output
    out     [4, 4, 1024, 8, 1024] f32
where per (b, n, head h):
    xh   = x[b, n, :, h*64:(h+1)*64]            # [1024, 64]
    a2   = attn_w[b, n, h, :, 64:128]           # [1024, 64]
    e    = a2 @ xh.T                            # [1024, 1024]
    out[b, n, i, h, j] = softmax_j(e_self[i] + e[i, j]) = softmax_j(e[i, j])
(e_self is constant along the softmax axis so it cancels; a1 is never needed).

Sharding: the 16 (b, n) slabs are split 2-per-core across 8 NeuronCores
(pure data parallel, no collectives).
"""

import sys

sys.path.insert(0, "/opt/trn_rl_repo")

from contextlib import ExitStack

import numpy as np

import concourse.bass as bass
import concourse.tile as tile
from concourse import mybir
from concourse.masks import make_identity

NUM_CORES = 8
SLABS_PER_CORE = 2  # (b, n) pairs per core
N_C = 1024  # complexes
D = 512
H = 8  # heads
DH = 64  # head dim
NIB = N_C // 128  # i-blocks per slab

F32 = mybir.dt.float32
F32R = mybir.dt.float32r

# score matmuls in float32r (full 4-byte operands, 1 cycle/row for N>=256)
USE_F32R = True
BF16 = mybir.dt.bfloat16

# Engine-isolation probe for bench builds (no NTFF traces under this axon
# client). One of: "" (full), "noact" (skip exp), "noout" (skip output DMA),
# "nodve" (skip accum/recip/mul, DMA the exp tile), "nomm" (skip matmuls).
PROBE = ""

# A/B knob (cost-model makespan preferred per-head output DMAs over
# [128, 2048] head-pair batched ones; kept for future experiments)
OUT_BATCH = False

# Where the softmax row-sum is accumulated:
#   "dve"   - DVE tensor_scalar copy-pass with accum_out (4x mode per model)
#   "act"   - ACT activation accum_out (costs an accumulator-read per exp)
#   "mixed" - alternate per i-block to balance both engines
ACCUM_MODE = "dve"

# Matmul operand dtype: "f32r" (exact, 1 cyc/row per cost model) or "fp16"
# (hedge in case f32r's fast path doesn't hold on real HW; ~0.8% out err)
MM_DTYPE = "f32r"


def make_pools(ctx: ExitStack, tc: tile.TileContext):
    nc = tc.nc
    pools = {}
    pools["const"] = ctx.enter_context(tc.tile_pool(name="const", bufs=1))
    pools["xstage"] = ctx.enter_context(tc.tile_pool(name="xstage", bufs=2))
    pools["xT"] = ctx.enter_context(tc.tile_pool(name="xT", bufs=2))
    pools["a2stage"] = ctx.enter_context(tc.tile_pool(name="a2stage", bufs=2))
    pools["a2T"] = ctx.enter_context(tc.tile_pool(name="a2T", bufs=2))
    pools["exp"] = ctx.enter_context(tc.tile_pool(name="exp", bufs=16))
    pools["outp"] = ctx.enter_context(tc.tile_pool(name="outp", bufs=10))
    pools["stat"] = ctx.enter_context(tc.tile_pool(name="stat", bufs=4))
    pools["scratchp"] = ctx.enter_context(tc.tile_pool(name="scratchp", bufs=1))
    pools["tpsum"] = ctx.enter_context(tc.tile_pool(name="tpsum", bufs=2, space="PSUM"))
    pools["spsum"] = ctx.enter_context(tc.tile_pool(name="spsum", bufs=3, space="PSUM"))
    identity = pools["const"].tile([128, 128], F32)
    make_identity(nc, identity[:])
    pools["identity"] = identity
    # dead sink for the DVE accumulate pass (only accum_out matters)
    pools["scratch"] = pools["scratchp"].tile([128, N_C], BF16, name="scratch")
    if PROBE:
        # probe builds read tiles that are never produced; give them a
        # defined source so Tile dependency tracking stays happy
        nc.vector.memset(pools["scratch"][:], 1.0)
        pools["psrc"] = pools["scratchp"].tile(
            [128, N_C], BF16, name="psrc", tag="psrc"
        )
        nc.vector.memset(pools["psrc"][:], 1.0)
    return pools


def build_kernel_body(pools, tc: tile.TileContext, out_ap, x_ap, w_ap):
    nc = tc.nc
    if PROBE == "empty":
        return
    xstage = pools["xstage"]
    xT_pool = pools["xT"]
    a2stage = pools["a2stage"]
    a2T_pool = pools["a2T"]
    exp_pool = pools["exp"]
    outp = pools["outp"]
    stat_pool = pools["stat"]
    tpsum = pools["tpsum"]
    spsum = pools["spsum"]
    identity = pools["identity"]

    if MM_DTYPE == "fp16":
        mm_dt = mybir.dt.float16
    else:
        mm_dt = F32R if USE_F32R else F32

    def load_a2(s, q, split=False):
        # a2s[:, ib*128 + hh*64 + k] = a2[s, 2q+hh, ib*128 + i, k]
        # (w_ap holds only the a2 half of attn_w, sliced host-side, so the
        # source rows are fully contiguous)
        # split=True loads i-halves as separate DMAs so the first i-block
        # transposes can start after a quarter of the bytes land
        a2s = a2stage.tile([128, N_C], F32, name="a2s", tag="a2s")
        a2s_r = a2s[:].rearrange("p (a c) -> p a c", c=128)
        halves = ((0, N_C // 2), (N_C // 2, N_C)) if split else ((0, N_C),)
        for i0, i1 in halves:
            a0, a1 = i0 // 128, i1 // 128
            for hh in range(2):
                h = 2 * q + hh
                src = w_ap[s, h, i0:i1, :].rearrange(
                    "(a p) k -> p a k", p=128
                )
                nc.gpsimd.dma_start(a2s_r[:, a0:a1, hh * DH : (hh + 1) * DH], src)
        return a2s

    a2s_next = None
    for s in range(SLABS_PER_CORE):
        # first head-pair's a2 load goes ahead of the bulk x load so the
        # a2 transposes (which gate the first scores) aren't queued behind
        # 2 MB of x on the DMA engines
        if a2s_next is None:
            a2s_next = load_a2(s, 0, split=True)
        # ---- x[s]: [1024 (j), 512 (d)] -> xT [512 (d), 1024 (j)] ----
        # xT packed as one [128, 4096] tile: xT[dd, p*1024 + j] = x[s, j, p*128+dd]
        # Loaded per 128-wide d-block (p): one DMA brings every j row's
        # p-th 512 B chunk.  The p-block's transposes are emitted inside
        # the q loop (p == q) so PE's in-order stream doesn't queue all 32
        # transposes ahead of the first score matmuls.
        xT = xT_pool.tile([128, 4 * N_C], mm_dt)
        xps = []
        for p in range(4):
            xp = xstage.tile([128, NIB * 128], F32, name="xp", tag="xp", bufs=6)
            # first-ever p-block in j-halves (startup fine-graining)
            halves = ((0, N_C // 2), (N_C // 2, N_C)) if (s == 0 and p == 0) else (
                (0, N_C),
            )
            for j0, j1 in halves:
                src = x_ap[s][j0:j1, p * 128 : (p + 1) * 128].rearrange(
                    "(a pp) d -> pp a d", pp=128
                )
                nc.gpsimd.dma_start(
                    xp[:, j0:j1].rearrange("pp (a d) -> pp a d", d=128), src
                )
            xps.append(xp)

        # ---- per head-pair q: heads (2q, 2q+1) ----
        for q in range(4):
            a2s = a2s_next
            # prefetch the next pair's a2 (next q, or q=0 of the next slab)
            if q < 3:
                a2s_next = load_a2(s, q + 1)
            elif s + 1 < SLABS_PER_CORE:
                a2s_next = load_a2(s + 1, 0)
            else:
                a2s_next = None

            # transpose to a2T [128 (k of pair), 1024 (i)]:
            # a2T[hh*64 + k, i] = a2 of head (2q+hh) at [i, k]
            # 4 transposes packed into one PSUM bank (only the first clears
            # the bank's has_written bits), drained by a single DVE copy.
            a2T = a2T_pool.tile([128, N_C], mm_dt)
            for ibq in range(NIB // 4):
                ps = tpsum.tile([128, 512], F32)
                for k in range(4):
                    ib = 4 * ibq + k
                    nc.tensor.matmul(
                        ps[:, k * 128 : (k + 1) * 128],
                        a2s[:, ib * 128 : (ib + 1) * 128],
                        identity[:],
                        is_transpose=True,
                        start=(k == 0),
                        stop=(k == 3),
                    )
                nc.vector.tensor_copy(
                    out=a2T[:, ibq * 512 : (ibq + 1) * 512], in_=ps[:]
                )

            # x transposes for this pair's d-block (p == q), same packing
            for jbq in range(NIB // 4):
                ps = tpsum.tile([128, 512], F32)
                for k in range(4):
                    jb = 4 * jbq + k
                    nc.tensor.matmul(
                        ps[:, k * 128 : (k + 1) * 128],
                        xps[q][:, jb * 128 : (jb + 1) * 128],
                        identity[:],
                        is_transpose=True,
                        start=(k == 0),
                        stop=(k == 3),
                    )
                nc.vector.tensor_copy(
                    out=xT[:, q * N_C + jbq * 512 : q * N_C + (jbq + 1) * 512],
                    in_=ps[:],
                )

            # ---- scores + softmax per head (two-phase over i-blocks) ----
            # Phase 1: scores -> exp (ACT, no accum read) + DVE accumulate
            # pass at 4x (bf16) collecting per-i row sums.  Phase 2: one
            # batched reciprocal, then DVE 4x normalize muls.  Output tiles
            # span the head pair ([128, 2048]); heads are adjacent in DRAM,
            # so one DMA per (q, ib) writes 4 KiB contiguous per partition
            # and halves the SP DGE-config cost per byte.
            outts = [
                outp.tile([128, 2 * N_C], BF16, name="outt", tag="outt")
                for _ in range(NIB)
            ] if OUT_BATCH else None
            for hh in range(2):
                h = 2 * q + hh
                # rhs: xT rows h*64..h*64+64 = partition offset hh*64 of block p=q
                rhs_all = xT[hh * DH : (hh + 1) * DH, q * N_C : (q + 1) * N_C]
                sums = stat_pool.tile([128, NIB], F32, tag="sums")
                rec = stat_pool.tile([128, NIB], F32, tag="rec")
                expts = []
                for ib in range(NIB):
                    lhsT = a2T[hh * DH : (hh + 1) * DH, ib * 128 : (ib + 1) * 128]
                    psc = spsum.tile([128, N_C], F32)
                    if PROBE != "nomm":
                        for jc in range(2):
                            nc.tensor.matmul(
                                psc[:, jc * 512 : (jc + 1) * 512],
                                lhsT,
                                rhs_all[:, jc * 512 : (jc + 1) * 512],
                                start=True,
                                stop=True,
                            )
                    on_act = ACCUM_MODE == "act" or (
                        ACCUM_MODE == "mixed" and ib % 2 == 0
                    )
                    if PROBE in ("noact", "nomm"):
                        expt = pools["psrc"]
                        on_act = False
                    else:
                        expt = exp_pool.tile([128, N_C], BF16)
                        if on_act and PROBE != "nodve":
                            nc.scalar.activation(
                                expt[:],
                                psc[:],
                                mybir.ActivationFunctionType.Exp,
                                accum_out=sums[:, ib : ib + 1],
                            )
                        else:
                            nc.scalar.activation(
                                expt[:], psc[:], mybir.ActivationFunctionType.Exp
                            )
                    if PROBE != "nodve" and not on_act:
                        nc.vector.tensor_scalar(
                            out=pools["scratch"][:],
                            in0=expt[:],
                            scalar1=1.0,
                            scalar2=None,
                            op0=mybir.AluOpType.mult,
                            op1=mybir.AluOpType.add,
                            accum_out=sums[:, ib : ib + 1],
                        )
                    expts.append(expt)
                if PROBE != "nodve":
                    # split so the first chunks' normalize muls can overlap
                    # the later accumulate passes; the final head uses
                    # quarters to shorten the end-of-kernel drain
                    last = s == SLABS_PER_CORE - 1 and q == 3 and hh == 1
                    step = NIB // 4 if last else NIB // 2
                    for c0 in range(0, NIB, step):
                        nc.vector.reciprocal(
                            rec[:, c0 : c0 + step], sums[:, c0 : c0 + step]
                        )
                for ib in range(NIB):
                    if PROBE == "nodve":
                        if PROBE != "noout":
                            # outts never written in this probe; ship exp tiles
                            nc.sync.dma_start(
                                out_ap[s, ib * 128 : (ib + 1) * 128, h, :],
                                expts[ib][:],
                            )
                        continue
                    if OUT_BATCH:
                        nc.vector.tensor_scalar_mul(
                            outts[ib][:, hh * N_C : (hh + 1) * N_C],
                            expts[ib][:],
                            rec[:, ib : ib + 1],
                        )
                        if hh == 1 and PROBE != "noout":
                            dst = out_ap[
                                s, ib * 128 : (ib + 1) * 128, 2 * q : 2 * q + 2, :
                            ].rearrange("p h j -> p (h j)")
                            nc.sync.dma_start(dst, outts[ib][:])
                    else:
                        outt = outp.tile(
                            [128, N_C], BF16, name="outs", tag="outs", bufs=8
                        )
                        nc.vector.tensor_scalar_mul(
                            outt[:], expts[ib][:], rec[:, ib : ib + 1]
                        )
                        if PROBE != "noout":
                            nc.sync.dma_start(
                                out_ap[s, ib * 128 : (ib + 1) * 128, h, :], outt[:]
                            )


def _split_multi_waits(nc):
    """walrus's per-instruction codegen structs hold only one embedded sync
    wait; hoist multi-wait instructions' waits onto standalone same-engine
    wait instructions placed immediately before them (program order on the
    sequencer preserves semantics)."""
    ctr = 0
    for f in nc.m.functions:
        for blk in f.blocks:
            out = []
            changed = False
            for inst in blk.instructions:
                tname = type(inst).__name__
                si = inst.sync_info
                if (
                    tname != "InstEventSemaphore"
                    and si is not None
                    and si.on_wait
                    and len(si.on_wait) > 1
                ):
                    for w in si.on_wait:
                        wi = mybir.InstEventSemaphore(name=f"WSPLIT-{ctr}")
                        ctr += 1
                        wi.engine = inst.engine
                        wi.sync_info = mybir.SyncInfo(on_wait=[w], on_update=[])
                        out.append(wi)
                    inst.sync_info = mybir.SyncInfo(
                        on_wait=[], on_update=list(si.on_update)
                    )
                    changed = True
                out.append(inst)
            if changed:
                blk.instructions = out
    return ctr


def build_bass(bench_repeats=None, split_waits=True):
    nc = bass.Bass("TRN2", target_bir_lowering=False, debug=False)
    if bench_repeats is None:
        x_ap = nc.dram_tensor(
            "x", [SLABS_PER_CORE, N_C, D], F32, kind="ExternalInput"
        ).ap()
        w_ap = nc.dram_tensor(
            "attn_w", [SLABS_PER_CORE, H, N_C, DH], F32, kind="ExternalInput"
        ).ap()
        out_ap = nc.dram_tensor(
            "out", [SLABS_PER_CORE, N_C, H, N_C], BF16, kind="ExternalOutput"
        ).ap()
        with tile.TileContext(nc) as tc:
            with ExitStack() as ctx:
                pools = make_pools(ctx, tc)
                build_kernel_body(pools, tc, out_ap, x_ap, w_ap)
    else:
        # bench variant: all big tensors are device-internal (no host I/O);
        # tiny external in/out keep the custom-call ABI happy. Internal
        # inputs are zeroed once, then the body runs `bench_repeats` times
        # (unrolled; For_i trips a walrus InstISA codegen bug).
        x_ap = nc.dram_tensor("xi", [SLABS_PER_CORE, N_C, D], F32).ap()
        w_ap = nc.dram_tensor("wi", [SLABS_PER_CORE, H, N_C, DH], F32).ap()
        out_ap = nc.dram_tensor("oi", [SLABS_PER_CORE, N_C, H, N_C], BF16).ap()
        tin = nc.dram_tensor("tin", [1, 4], F32, kind="ExternalInput").ap()
        tout = nc.dram_tensor("tout", [1, 4], F32, kind="ExternalOutput").ap()
        with tile.TileContext(nc) as tc:
            with ExitStack() as ctx:
                pools = make_pools(ctx, tc)
                tiny = pools["const"].tile([1, 4], F32)
                nc.gpsimd.dma_start(tiny[:], tin[:, :])
                nc.gpsimd.dma_start(tout[:, :], tiny[:])
                zt = pools["const"].tile([128, 4 * N_C], F32)
                nc.vector.memset(zt[:], 0.0)
                x_flat = x_ap.rearrange("s (a p) d -> (s a) p d", p=128)
                for t in range(x_flat.shape[0]):
                    nc.gpsimd.dma_start(x_flat[t], zt[:, :D])
                w_flat = w_ap.rearrange("s h (a p) k -> (s h a) p k", p=128)
                for t in range(w_flat.shape[0]):
                    nc.gpsimd.dma_start(w_flat[t], zt[:, :DH])
                for _ in range(bench_repeats):
                    build_kernel_body(pools, tc, out_ap, x_ap, w_ap)
    if split_waits:
        _split_multi_waits(nc)
    return nc


_NC_CACHE = None


def _get_nc():
    global _NC_CACHE
    if _NC_CACHE is None:
        _NC_CACHE = build_bass()
    return _NC_CACHE


def make_runner(nc, n_cores=NUM_CORES):
    """Build a cached jax-jitted SPMD callable for a finalized Bass module.

    Unlike run_bass_kernel_spmd (fresh closure -> jit cache miss -> full
    re-trace + XLA recompile on EVERY call), the returned callable is
    compiled once and dispatches through jax's fast path afterwards.
    Outputs are bound as custom-call results (neuronx_cc_hook renames BIR
    ExternalOutputs to output{i}, which libneuronpjrt binds to the result
    buffers), the same structure bass_jit emits — so no full-size zero
    output arrays are allocated on host or shipped over the tunnel per
    call, unlike run_bass_via_pjrt's donation path.  Requires the kernel
    to write every output element (ours does).

    Takes the global (n_cores*dim0-concatenated) numpy/jax arrays for each
    BIR ExternalInput in allocation order; returns global jax arrays for
    each ExternalOutput.
    """
    import jax
    from jax.sharding import Mesh, PartitionSpec

    from jax.experimental.shard_map import shard_map

    from concourse.bass2jax import (
        _bass_exec_p,
        install_neuronx_cc_hook,
        partition_id_tensor,
    )

    install_neuronx_cc_hook()
    partition_name = (
        nc.partition_id_tensor.name if nc.partition_id_tensor is not None else None
    )
    in_names, out_names, out_avals = [], [], []
    for alloc in nc.m.functions[0].allocations:
        if not isinstance(alloc, mybir.MemoryLocationSet):
            continue
        name = alloc.memorylocations[0].name
        if alloc.kind == "ExternalInput":
            if name != partition_name:
                in_names.append(name)
        elif alloc.kind == "ExternalOutput":
            out_avals.append(
                jax.core.ShapedArray(tuple(alloc.tensor_shape), mybir.dt.np(alloc.dtype))
            )
            out_names.append(name)
    all_in_names = tuple(in_names + ([partition_name] if partition_name else []))

    def _body(*args):
        operands = list(args)
        if partition_name:
            operands.append(partition_id_tensor())
        outs = _bass_exec_p.bind(
            *operands,
            out_avals=tuple(out_avals),
            in_names=all_in_names,
            out_names=tuple(out_names),
            lowering_input_output_aliases=(),
            sim_require_finite=True,
            sim_require_nnan=True,
            nc=nc,
        )
        return tuple(outs)

    devices = jax.devices()[:n_cores]
    mesh = Mesh(np.asarray(devices), ("core",))
    in_specs = (PartitionSpec("core"),) * len(in_names)
    out_specs = (PartitionSpec("core"),) * len(out_names)
    return jax.jit(
        shard_map(_body, mesh=mesh, in_specs=in_specs, out_specs=out_specs, check_rep=False)
    )


_RUNNER_CACHE = None


def _get_runner():
    global _RUNNER_CACHE
    if _RUNNER_CACHE is None:
        _RUNNER_CACHE = make_runner(_get_nc())
    return _RUNNER_CACHE


def kernel(x: np.ndarray, attn_w: np.ndarray, _trace: bool = False):
    assert x.shape == (4, 4, N_C, D), x.shape
    assert attn_w.shape == (4, 4, H, N_C, 2 * DH), attn_w.shape
    import jax

    xs = np.ascontiguousarray(x, dtype=np.float32).reshape(16, N_C, D)
    # only the a2 half of attn_w enters the scores (e_self is constant along
    # the softmax axis and cancels); slice host-side to halve the upload
    a2 = np.ascontiguousarray(
        np.asarray(attn_w, dtype=np.float32)[..., DH:]
    ).reshape(16, H, N_C, DH)
    fn = _get_runner()
    (out_g,) = fn(xs, a2)
    out = np.asarray(jax.block_until_ready(out_g))  # [16, N_C, H, N_C] bf16
    return out.astype(np.float32).reshape(4, 4, N_C, H, N_C)


kernel.last_exec_time_ns = None

